# revision 52
# baseline (speedup 1.0000x reference)
"""Trainium2 Bass kernel for a single attention head (B=4, S=2048, D=4096, DH=128).

Sharding: 8 cores = (batch b, parity h). Core (b, h) owns q-tiles {h, h+2, ...,
h+14} of its batch -- even/odd striping balances the causal triangle exactly
(2(i+1) key-chunks for the i-th q-tile, i=0..7).

Projection work is deduplicated across each core pair with an AllGather:
every core projects K/V/Q only for its OWN 1024 columns (x read once, 8.4MB),
in two 512-column groups; after each group the K/V halves are exchanged
through a DRAM bounce AllGather over the pair. K^T/V^T land in CANONICAL
[even tiles | odd tiles] order (replica order), which makes the readback
rank-independent. Group 1's collective flies under group 2's matmuls; group
2's flies under attention tiles 0-3 (which only need group-1 keys).

Attention per q-tile i: even chunks 0..i + odd chunks 0..i. Only two blocks
are mask-dependent (host supplies mask[h*128:(h+1)*128, 0:128] for the last
even chunk and [.., 128:256] for the last odd chunk -- by causal structure
these equal every such block). Softmax splits across engines: DVE adds the
two mask blocks in PSUM, row-max reduces both PSUM halves (negated f32 max
rounded to bf16 -- exact for bf16 logits since RNE is monotone), casts the
odd half; ACT casts the even half and does exp with a row-sum accumulator.
W^T is DMA-transposed into a pair-shared [128, slot, 256] buffer; PV runs
256-wide over q-tile pairs, output transposed. The V bias enters linearly
and is added on the host; the output is the unnormalized PV^T plus softmax
row sums, divided on the host.
"""

import numpy as np
import ml_dtypes

import concourse.tile as tile
from concourse import bacc, mybir
from concourse.bass_utils import run_bass_kernel_spmd

B, S, D, DH = 4, 2048, 4096, 128
SQ = S // 2          # q rows per core
N_CORES = 8
D_CH = D // 128      # 32 contraction chunks
QT = 8               # q row tiles per core
GROUPS = [[0, 1], [2, 3], [4, 5], [6, 7]]

BF16 = mybir.dt.bfloat16
F32 = mybir.dt.float32


def build_nc():
    nc = bacc.Bacc(None)

    xT = nc.dram_tensor("xT", [D, SQ], BF16, kind="ExternalInput")
    # weights pre-tiled on host: w[p, i, m] = W[m, i*128+p]
    wqT = nc.dram_tensor("wqT", [128, D_CH, DH], BF16, kind="ExternalInput")
    wkT = nc.dram_tensor("wkT", [128, D_CH, DH], BF16, kind="ExternalInput")
    wvT = nc.dram_tensor("wvT", [128, D_CH, DH], BF16, kind="ExternalInput")
    bq = nc.dram_tensor("bq", [DH, 1], F32, kind="ExternalInput")
    bk = nc.dram_tensor("bk", [DH, 1], F32, kind="ExternalInput")
    maskA = nc.dram_tensor("maskA", [128, 128], BF16, kind="ExternalInput")
    maskB = nc.dram_tensor("maskB", [128, 128], BF16, kind="ExternalInput")
    outT = nc.dram_tensor("outT", [DH, SQ], BF16, kind="ExternalOutput")
    sums = nc.dram_tensor("sums", [128, QT], F32, kind="ExternalOutput")

    with tile.TileContext(nc) as tc:
        with (
            tc.tile_pool(name="weights", bufs=1) as wpool,
            tc.tile_pool(name="persist", bufs=1) as persist,
            tc.tile_pool(name="xres", bufs=D_CH) as xres,
            tc.tile_pool(name="stage", bufs=2) as stpool,
            tc.tile_pool(name="dram", bufs=2, space="DRAM") as dram,
        ):
            w_sb = {}
            for name in ("q", "k", "v"):
                w_sb[name] = wpool.tile([128, D_CH, DH], BF16, tag=f"w{name}",
                                        name=f"w{name}")
            b_sb = {}
            for name in ("k", "q"):
                b_sb[name] = wpool.tile([DH, 1], F32, tag=f"b{name}",
                                        name=f"b{name}")
            mA_sb = persist.tile([128, 128], BF16, tag="mA")
            mB_sb = persist.tile([128, 128], BF16, tag="mB")
            # dummy first DVE op: absorbs the scheduler's padded first-wait so
            # the group-0 K/V evacuation gets a tight semaphore target
            scratch = persist.tile([128, 1], F32, tag="scratch")
            nc.vector.memset(scratch[:], 0.0)
            # K/V weights first on the scalar queue; x odd chunks follow on
            # the same queue (evens on sync); Q weights + biases + masks are
            # deferred behind the x stream (Q projections start much later)
            for sl in range(4):
                for name, ext in (("k", wkT), ("v", wvT)):
                    ss = np.s_[:, sl * 8:(sl + 1) * 8, :]
                    nc.scalar.dma_start(out=w_sb[name][ss], in_=ext[ss])

            def late_weights():
                for sl in range(4):
                    ss = np.s_[:, sl * 8:(sl + 1) * 8, :]
                    nc.scalar.dma_start(out=w_sb["q"][ss], in_=wqT[ss])
                nc.scalar.dma_start(out=b_sb["k"][:], in_=bk[:])
                nc.scalar.dma_start(out=b_sb["q"][:], in_=bq[:])
                nc.scalar.dma_start(out=mA_sb[:], in_=maskA[:])
                nc.scalar.dma_start(out=mB_sb[:], in_=maskB[:])

            # K^T/V^T/Q^T and the V chunks are SPLIT per (parity r, column
            # group g) so consumers only depend on the collective that
            # actually produced their data (whole-tile dependency tracking
            # would otherwise serialize everything behind the last exchange)
            kt4 = {(r, g): persist.tile([DH, 512], BF16, tag=f"kt{r}{g}",
                                        name=f"kt{r}{g}")
                   for r in range(2) for g in range(2)}
            vt4 = {(r, g): persist.tile([DH, 512], BF16, tag=f"vt{r}{g}",
                                        name=f"vt{r}{g}")
                   for r in range(2) for g in range(2)}
            v4 = {(r, g): persist.tile([128, 4, DH], BF16, tag=f"v{r}{g}",
                                       name=f"v{r}{g}")
                  for r in range(2) for g in range(2)}
            qt2 = {g: persist.tile([DH, 512], BF16, tag=f"qt{g}",
                                   name=f"qt{g}")
                   for g in range(2)}
            sums_sb = persist.tile([128, QT], F32, tag="sums")

            xts = []

            def exchange_kv(acc):
                """Evacuate K/V, one AllGather across the pair, read back
                into canonical kt/vt, and interleave V into v chunks."""
                # the K bias adds a per-row constant to the logits, which
                # softmax cancels -- so it is dropped and both evacuations
                # are pure casts (on ACT, whose wait targets schedule tight)
                stage = stpool.tile([128, 2048], BF16, tag="stage")
                for g in range(2):
                    nc.scalar.copy(stage[:, g * 512:(g + 1) * 512],
                                   acc[f"k{g}"][:])
                    nc.scalar.copy(stage[:, 1024 + g * 512:1024 + (g + 1) * 512],
                                   acc[f"v{g}"][:])
                inb = dram.tile([128, 2048], BF16, tag="inb")
                outb = dram.tile([2, 128, 2048], BF16, tag="outb")
                nc.gpsimd.dma_start(out=inb[:], in_=stage[:])
                nc.gpsimd.collective_compute(
                    "AllGather", mybir.AluOpType.bypass,
                    replica_groups=GROUPS, ins=[inb.opt()], outs=[outb.opt()])
                for r in range(2):
                    for g in range(2):
                        nc.scalar.dma_start(out=kt4[(r, g)][:],
                                            in_=outb[r, :, g * 512:(g + 1) * 512])
                        nc.scalar.dma_start(
                            out=vt4[(r, g)][:],
                            in_=outb[r, :, 1024 + g * 512:1024 + (g + 1) * 512])
                        nc.sync.dma_start_transpose(out=v4[(r, g)][:],
                                                    in_=vt4[(r, g)][:])

            # --- projections: K/V first (collectives fly early), Q under
            # the collective flights, all from one SBUF-resident x read ---
            with (
                tc.tile_pool(name="psum_kv0", bufs=1, space="PSUM") as pkv0,
                tc.tile_pool(name="psum_kv1", bufs=1, space="PSUM") as pkv1,
                tc.tile_pool(name="psum_q", bufs=1, space="PSUM") as pq,
            ):
                akv = {f"{t}{g}": p.tile([DH, 512], F32, tag=f"kv{g}{t}",
                                         name=f"kv{g}{t}")
                       for t in ("k", "v")
                       for g, p in ((0, pkv0), (1, pkv1))}
                aq = [pq.tile([DH, 512], F32, tag=f"q{g}", name=f"q{g}")
                      for g in range(2)]

                def q_proj(g):
                    for i in range(D_CH):
                        st = dict(start=(i == 0), stop=(i == D_CH - 1))
                        cs = np.s_[:, g * 512:(g + 1) * 512]
                        nc.tensor.matmul(aq[g][:], lhsT=w_sb["q"][:, i, :],
                                         rhs=xts[i][cs], **st)
                    nc.vector.tensor_scalar_add(qt2[g][:], aq[g][:], b_sb["q"][:])

                for i in range(D_CH):
                    xt = xres.tile([128, SQ], BF16, tag="xt", name="xt")
                    xts.append(xt)
                    q = nc.sync if i % 2 == 0 else nc.scalar
                    q.dma_start(out=xt[:], in_=xT[i * 128:(i + 1) * 128, :])
                    st = dict(start=(i == 0), stop=(i == D_CH - 1))
                    for g in range(2):
                        cs = np.s_[:, g * 512:(g + 1) * 512]
                        nc.tensor.matmul(akv[f"k{g}"][:], lhsT=w_sb["k"][:, i, :],
                                         rhs=xt[cs], **st)
                        nc.tensor.matmul(akv[f"v{g}"][:], lhsT=w_sb["v"][:, i, :],
                                         rhs=xt[cs], **st)
                exchange_kv(akv)
                # Q weights load only now (dispatched behind the exchange
                # casts on ACT): a real data dependency that stops the
                # scheduler from hoisting Q matmuls into the K/V stream,
                # so the collective triggers as early as possible
                late_weights()
                q_proj(0)  # Q fills the PE while the collective flies
                q_proj(1)

            # --- attention, software-pipelined over q-tiles ---
            with (
                tc.tile_pool(name="lm_sb", bufs=2) as lmpool,
                tc.tile_pool(name="w_sb2", bufs=2) as wepool,
                tc.tile_pool(name="wt_sb", bufs=2) as wtpool,
                tc.tile_pool(name="o_sb", bufs=2) as opool,
                tc.tile_pool(name="stats", bufs=8) as stat,
                tc.tile_pool(name="l_psum", bufs=1, space="PSUM") as lpool,
                tc.tile_pool(name="o_psum", bufs=2, space="PSUM") as popool,
            ):
                pl_ev = lpool.tile([128, QT, 128], F32, tag="pl_ev",
                                   name="pl_ev")
                pl_od = lpool.tile([128, QT, 128], F32, tag="pl_od",
                                   name="pl_od")
                pair_bufs = {}
                tile_state = {}

                def stage1(i):
                    """logits -> mask adds -> row maxes -> bf16 casts. Emitted
                    one tile AHEAD of stage2 so the ACT queue's exp(i-1) stall
                    (waiting on the row max) doesn't delay cast-ev(i), which
                    releases the logits PSUM for tile i+1."""
                    e = (i + 1) * 128
                    qt_t = qt2[i // 4]
                    qsl = np.s_[:, (i % 4) * 128:(i % 4) * 128 + 128]
                    for lo in range(0, e, 512):
                        w = min(512, e - lo)
                        nc.tensor.matmul(pl_ev[:, lo // 128:(lo + w) // 128, :],
                                         lhsT=qt_t[qsl],
                                         rhs=kt4[(0, lo // 512)][:, 0:w],
                                         start=True, stop=True)
                    # the last even chunk is the only mask-dependent one
                    # (diagonal for h=0 cores, fully visible for h=1)
                    nc.vector.tensor_add(pl_ev[:, i, :], pl_ev[:, i, :],
                                         mA_sb[:])
                    nmo = stat.tile([128, 1], F32, tag="nmo")
                    nc.vector.reduce_max(out=nmo[:], in_=pl_ev[:, :i + 1, :],
                                         axis=mybir.AxisListType.XY, negate=True)
                    lm = lmpool.tile([128, S], BF16, tag="lm")
                    nc.scalar.copy(lm[:, :e], pl_ev[:, :i + 1, :])
                    for lo in range(0, e, 512):
                        w = min(512, e - lo)
                        nc.tensor.matmul(pl_od[:, lo // 128:(lo + w) // 128, :],
                                         lhsT=qt_t[qsl],
                                         rhs=kt4[(1, lo // 512)][:, 0:w],
                                         start=True, stop=True)
                    # ... and the last odd chunk (masked for h=0, diag for h=1)
                    nc.vector.tensor_add(pl_od[:, i, :], pl_od[:, i, :],
                                         mB_sb[:])
                    nmp = stat.tile([128, 1], F32, tag="nmp")
                    nc.vector.reduce_max(out=nmp[:], in_=pl_od[:, :i + 1, :],
                                         axis=mybir.AxisListType.XY, negate=True)
                    # combined -(row max), rounded to bf16: RNE is monotone, so
                    # this is exactly -(max of the bf16 logits)
                    nmb = stat.tile([128, 1], BF16, tag="nmb")
                    nc.vector.tensor_tensor(nmb[:], nmo[:], nmp[:],
                                            op=mybir.AluOpType.min)
                    nmf = stat.tile([128, 1], F32, tag="nmf")
                    nc.vector.tensor_copy(nmf[:], nmb[:])
                    nc.vector.tensor_copy(lm[:, e:2 * e], pl_od[:, :i + 1, :])
                    tile_state[i] = (lm, nmf)

                def stage2(i):
                    e = (i + 1) * 128
                    lm, nmf = tile_state.pop(i)
                    p = i // 2
                    if i % 2 == 0:
                        wt = wtpool.tile([128, 16, 256], BF16, tag="wt")
                        pair_bufs[p] = wt
                        # slots the even tile doesn't cover (its half only)
                        nc.gpsimd.memset(wt[:, i + 1, 0:128], 0.0)
                        nc.gpsimd.memset(wt[:, 9 + i, 0:128], 0.0)
                    else:
                        wt = pair_bufs[p]
                    half = np.s_[(i % 2) * 128:(i % 2) * 128 + 128]

                    w_t = wepool.tile([128, S], BF16, tag="w")
                    nc.scalar.activation(
                        out=w_t[:, :2 * e], in_=lm[:, :2 * e],
                        func=mybir.ActivationFunctionType.Exp,
                        bias=nmf[:], scale=1.0,
                        accum_out=sums_sb[:, i:i + 1])

                    nc.sync.dma_start_transpose(out=wt[:, 0:i + 1, half],
                                                in_=w_t[:, :e])
                    nc.sync.dma_start_transpose(out=wt[:, 8:9 + i, half],
                                                in_=w_t[:, e:2 * e])

                def pv(p):
                    wt = pair_bufs.pop(p)
                    hi = 2 * p + 1
                    slots = list(range(0, hi + 1)) + list(range(8, 9 + hi))
                    po = popool.tile([128, 256], F32, tag="poT")
                    for n, s in enumerate(slots):
                        r, ls = s // 8, s % 8
                        vt_t = v4[(r, ls // 4)]
                        nc.tensor.matmul(po[:], lhsT=vt_t[:, ls % 4, :],
                                         rhs=wt[:, s, :],
                                         start=(n == 0), stop=(n == len(slots) - 1))
                    o_sb = opool.tile([128, 256], BF16, tag="o")
                    nc.vector.tensor_copy(o_sb[:], po[:])
                    nc.gpsimd.dma_start(out=outT[:, p * 256:(p + 1) * 256],
                                        in_=o_sb[:])

                # lag-1 schedule: stage1 runs one tile ahead of stage2
                stage1(0)
                for i in range(1, QT):
                    stage1(i)
                    stage2(i - 1)
                    if i % 2 == 0 and i >= 2:
                        pv(i // 2 - 1)
                stage2(QT - 1)
                pv(3)
                nc.gpsimd.dma_start(out=sums[:], in_=sums_sb[:])

    nc.finalize()
    return nc


def shard_inputs(x, attn_mask, Wq, bq, Wk, bk, Wv, bv):
    """Host-side shard prep. Returns in_maps for cores 0..7."""
    bf = ml_dtypes.bfloat16
    xb = np.asarray(x).astype(bf)                   # cast first, like the reference
    mask_f = np.asarray(attn_mask)

    def tile_w(W):
        # [DH, D] -> [128, D_CH, DH] with w[p, i, m] = W[m, i*128+p]
        WT = np.asarray(W).astype(bf).T.reshape(D_CH, 128, DH)
        return np.ascontiguousarray(WT.transpose(1, 0, 2))

    wqt, wkt, wvt = tile_w(Wq), tile_w(Wk), tile_w(Wv)
    bqc = np.asarray(bq).astype(bf).astype(np.float32).reshape(DH, 1)
    bkc = np.asarray(bk).astype(bf).astype(np.float32).reshape(DH, 1)

    in_maps = []
    for c in range(N_CORES):
        b, h = divmod(c, 2)
        own = np.concatenate([np.arange(t * 128, (t + 1) * 128)
                              for t in range(h, 16, 2)])
        xT = np.ascontiguousarray(xb[b][own].T)                      # [D, SQ]
        # by causal structure every (q-tile, last even/odd chunk) mask block
        # equals the corresponding block in the first two tile rows
        mA = np.ascontiguousarray(mask_f[h * 128:(h + 1) * 128, 0:128].astype(bf))
        mB = np.ascontiguousarray(mask_f[h * 128:(h + 1) * 128, 128:256].astype(bf))
        in_maps.append({
            "xT": xT, "maskA": mA, "maskB": mB,
            "wqT": wqt, "wkT": wkt, "wvT": wvt,
            "bq": bqc, "bk": bkc,
        })
    return in_maps


def unshard(core_out, bv):
    """core_out: dicts with 'outT' [DH, SQ] bf16, 'sums' [128, QT] f32."""
    bvf = np.asarray(bv).astype(ml_dtypes.bfloat16).astype(np.float32)
    out = np.empty((B, S, DH), dtype=ml_dtypes.bfloat16)
    for c in range(N_CORES):
        b, h = divmod(c, 2)
        oT = np.asarray(core_out[c]["outT"], dtype=np.float32)
        sm = np.asarray(core_out[c]["sums"], dtype=np.float32)
        for j in range(QT):
            t = h + 2 * j
            blk = oT[:, j * 128:(j + 1) * 128] / sm[:, j][None, :]
            out[b, t * 128:(t + 1) * 128, :] = \
                (blk.T + bvf[None, :]).astype(ml_dtypes.bfloat16)
    return out


_NC_CACHE = {}


def kernel(x, attn_mask, Wq, bq, Wk, bk, Wv, bv):
    if "nc" not in _NC_CACHE:
        _NC_CACHE["nc"] = build_nc()
    nc = _NC_CACHE["nc"]
    in_maps = shard_inputs(x, attn_mask, Wq, bq, Wk, bk, Wv, bv)
    res = run_bass_kernel_spmd(nc, in_maps, list(range(N_CORES)))
    return unshard(res.results, bv)


# revision 53
# speedup vs baseline: 1.1046x; 1.1046x over previous
"""Trainium2 Bass kernel for a single attention head (B=4, S=2048, D=4096, DH=128).

Sharding: 8 cores = (batch b, parity h). Core (b, h) owns q-tiles {h, h+2, ...,
h+14} of its batch -- even/odd striping balances the causal triangle exactly
(2(i+1) key-chunks for the i-th q-tile, i=0..7).

Projection work is deduplicated across each core pair with an AllGather:
every core projects K/V/Q only for its OWN 1024 columns (x read once, 8.4MB),
in two 512-column groups; after each group the K/V halves are exchanged
through a DRAM bounce AllGather over the pair. K^T/V^T land in CANONICAL
[even tiles | odd tiles] order (replica order), which makes the readback
rank-independent. Group 1's collective flies under group 2's matmuls; group
2's flies under attention tiles 0-3 (which only need group-1 keys).

Attention per q-tile i: even chunks 0..i + odd chunks 0..i. Only two blocks
are mask-dependent (host supplies mask[h*128:(h+1)*128, 0:128] for the last
even chunk and [.., 128:256] for the last odd chunk -- by causal structure
these equal every such block). Softmax splits across engines: DVE adds the
two mask blocks in PSUM, row-max reduces both PSUM halves (negated f32 max
rounded to bf16 -- exact for bf16 logits since RNE is monotone), casts the
odd half; ACT casts the even half and does exp with a row-sum accumulator.
W^T is DMA-transposed into a pair-shared [128, slot, 256] buffer; PV runs
256-wide over q-tile pairs, output transposed. The V bias enters linearly
and is added on the host; the output is the unnormalized PV^T plus softmax
row sums, divided on the host.
"""

import numpy as np
import ml_dtypes

import concourse.tile as tile
from concourse import bacc, mybir
from concourse.bass_utils import run_bass_kernel_spmd

B, S, D, DH = 4, 2048, 4096, 128
SQ = S // 2          # q rows per core
N_CORES = 8
D_CH = D // 128      # 32 contraction chunks
QT = 8               # q row tiles per core
GROUPS = [[0, 1], [2, 3], [4, 5], [6, 7]]

BF16 = mybir.dt.bfloat16
F32 = mybir.dt.float32


def build_nc():
    nc = bacc.Bacc(None)

    xT = nc.dram_tensor("xT", [D, SQ], BF16, kind="ExternalInput")
    # weights pre-tiled on host: w[p, i, m] = W[m, i*128+p]
    wqT = nc.dram_tensor("wqT", [128, D_CH, DH], BF16, kind="ExternalInput")
    wkT = nc.dram_tensor("wkT", [128, D_CH, DH], BF16, kind="ExternalInput")
    wvT = nc.dram_tensor("wvT", [128, D_CH, DH], BF16, kind="ExternalInput")
    bq = nc.dram_tensor("bq", [DH, 1], F32, kind="ExternalInput")
    bk = nc.dram_tensor("bk", [DH, 1], F32, kind="ExternalInput")
    maskA = nc.dram_tensor("maskA", [128, 128], BF16, kind="ExternalInput")
    maskB = nc.dram_tensor("maskB", [128, 128], BF16, kind="ExternalInput")
    outT = nc.dram_tensor("outT", [DH, SQ], BF16, kind="ExternalOutput")
    sums = nc.dram_tensor("sums", [128, QT], F32, kind="ExternalOutput")

    with tile.TileContext(nc) as tc:
        with (
            tc.tile_pool(name="weights", bufs=1) as wpool,
            tc.tile_pool(name="persist", bufs=1) as persist,
            tc.tile_pool(name="xres", bufs=D_CH) as xres,
            tc.tile_pool(name="stage", bufs=2) as stpool,
            tc.tile_pool(name="dram", bufs=2, space="DRAM") as dram,
        ):
            w_sb = {}
            for name in ("q", "k", "v"):
                w_sb[name] = wpool.tile([128, D_CH, DH], BF16, tag=f"w{name}",
                                        name=f"w{name}")
            b_sb = {}
            for name in ("k", "q"):
                b_sb[name] = wpool.tile([DH, 1], F32, tag=f"b{name}",
                                        name=f"b{name}")
            mA_sb = persist.tile([128, 128], BF16, tag="mA")
            mB_sb = persist.tile([128, 128], BF16, tag="mB")
            # dummy first DVE op: absorbs the scheduler's padded first-wait so
            # the group-0 K/V evacuation gets a tight semaphore target
            scratch = persist.tile([128, 1], F32, tag="scratch")
            nc.vector.memset(scratch[:], 0.0)
            # K/V weights first on the scalar queue; x odd chunks follow on
            # the same queue (evens on sync); Q weights + biases + masks are
            # deferred behind the x stream (Q projections start much later)
            for sl in range(4):
                for name, ext in (("k", wkT), ("v", wvT)):
                    ss = np.s_[:, sl * 8:(sl + 1) * 8, :]
                    nc.scalar.dma_start(out=w_sb[name][ss], in_=ext[ss])

            def late_weights():
                for sl in range(4):
                    ss = np.s_[:, sl * 8:(sl + 1) * 8, :]
                    nc.scalar.dma_start(out=w_sb["q"][ss], in_=wqT[ss])
                nc.scalar.dma_start(out=b_sb["k"][:], in_=bk[:])
                nc.scalar.dma_start(out=b_sb["q"][:], in_=bq[:])
                nc.scalar.dma_start(out=mA_sb[:], in_=maskA[:])
                nc.scalar.dma_start(out=mB_sb[:], in_=maskB[:])

            # K^T/V^T/Q^T and the V chunks are SPLIT per (parity r, column
            # group g) so consumers only depend on the collective that
            # actually produced their data (whole-tile dependency tracking
            # would otherwise serialize everything behind the last exchange)
            kt4 = {(r, g): persist.tile([DH, 512], BF16, tag=f"kt{r}{g}",
                                        name=f"kt{r}{g}")
                   for r in range(2) for g in range(2)}
            vt4 = {(r, g): persist.tile([DH, 512], BF16, tag=f"vt{r}{g}",
                                        name=f"vt{r}{g}")
                   for r in range(2) for g in range(2)}
            v4 = {(r, g): persist.tile([128, 4, DH], BF16, tag=f"v{r}{g}",
                                       name=f"v{r}{g}")
                  for r in range(2) for g in range(2)}
            qt2 = {g: persist.tile([DH, 512], BF16, tag=f"qt{g}",
                                   name=f"qt{g}")
                   for g in range(2)}
            sums_sb = persist.tile([128, QT], F32, tag="sums")

            xts = []

            def exchange_kv(acc, g):
                """Evacuate group g's K/V, AllGather across the pair, read
                back into canonical kt/vt, and interleave V into v chunks."""
                # the K bias adds a per-row constant to the logits, which
                # softmax cancels -- so it is dropped and both evacuations
                # are pure casts (on ACT, whose wait targets schedule tight)
                stage = stpool.tile([128, 1024], BF16, tag="stage")
                nc.scalar.copy(stage[:, 0:512], acc["k"][:])
                nc.scalar.copy(stage[:, 512:1024], acc["v"][:])
                inb = dram.tile([128, 1024], BF16, tag="inb")
                outb = dram.tile([2, 128, 1024], BF16, tag="outb")
                nc.gpsimd.dma_start(out=inb[:], in_=stage[:])
                nc.gpsimd.collective_compute(
                    "AllGather", mybir.AluOpType.bypass,
                    replica_groups=GROUPS, ins=[inb.opt()], outs=[outb.opt()])
                for r in range(2):
                    nc.scalar.dma_start(out=kt4[(r, g)][:], in_=outb[r, :, 0:512])
                    nc.scalar.dma_start(out=vt4[(r, g)][:], in_=outb[r, :, 512:1024])
                    nc.sync.dma_start_transpose(out=v4[(r, g)][:],
                                                in_=vt4[(r, g)][:])

            # --- projections: K/V first (collectives fly early), Q under
            # the collective flights, all from one SBUF-resident x read ---
            with (
                tc.tile_pool(name="psum_kv0", bufs=1, space="PSUM") as pkv0,
                tc.tile_pool(name="psum_kv1", bufs=1, space="PSUM") as pkv1,
                tc.tile_pool(name="psum_q", bufs=1, space="PSUM") as pq,
            ):
                akv = [{t: p.tile([DH, 512], F32, tag=f"kv{g}{t}",
                                  name=f"kv{g}{t}")
                        for t in ("k", "v")}
                       for g, p in ((0, pkv0), (1, pkv1))]
                aq = [pq.tile([DH, 512], F32, tag=f"q{g}", name=f"q{g}")
                      for g in range(2)]
                def q_proj(g):
                    for i in range(D_CH):
                        st = dict(start=(i == 0), stop=(i == D_CH - 1))
                        cs = np.s_[:, g * 512:(g + 1) * 512]
                        nc.tensor.matmul(aq[g][:], lhsT=w_sb["q"][:, i, :],
                                         rhs=xts[i][cs], **st)
                    nc.vector.tensor_scalar_add(qt2[g][:], aq[g][:], b_sb["q"][:])

                for g in range(2):
                    for i in range(D_CH):
                        if g == 0:
                            xt = xres.tile([128, SQ], BF16, tag="xt", name="xt")
                            xts.append(xt)
                            q = nc.sync if i % 2 == 0 else nc.scalar
                            q.dma_start(out=xt[:],
                                        in_=xT[i * 128:(i + 1) * 128, :])
                            if i == D_CH - 1:
                                late_weights()
                        st = dict(start=(i == 0), stop=(i == D_CH - 1))
                        cs = np.s_[:, g * 512:(g + 1) * 512]
                        nc.tensor.matmul(akv[g]["k"][:], lhsT=w_sb["k"][:, i, :],
                                         rhs=xts[i][cs], **st)
                        nc.tensor.matmul(akv[g]["v"][:], lhsT=w_sb["v"][:, i, :],
                                         rhs=xts[i][cs], **st)
                    exchange_kv(akv[g], g)
                q_proj(0)  # Q fills the PE while the collectives fly
                q_proj(1)

            # --- attention, software-pipelined over q-tiles ---
            with (
                tc.tile_pool(name="lm_sb", bufs=2) as lmpool,
                tc.tile_pool(name="w_sb2", bufs=2) as wepool,
                tc.tile_pool(name="wt_sb", bufs=2) as wtpool,
                tc.tile_pool(name="o_sb", bufs=2) as opool,
                tc.tile_pool(name="stats", bufs=8) as stat,
                tc.tile_pool(name="l_psum", bufs=1, space="PSUM") as lpool,
                tc.tile_pool(name="o_psum", bufs=2, space="PSUM") as popool,
            ):
                pl_ev = lpool.tile([128, QT, 128], F32, tag="pl_ev",
                                   name="pl_ev")
                pl_od = lpool.tile([128, QT, 128], F32, tag="pl_od",
                                   name="pl_od")
                pair_bufs = {}
                tile_state = {}

                def stage1(i):
                    """logits -> mask adds -> row maxes -> bf16 casts. Emitted
                    one tile AHEAD of stage2 so the ACT queue's exp(i-1) stall
                    (waiting on the row max) doesn't delay cast-ev(i), which
                    releases the logits PSUM for tile i+1."""
                    e = (i + 1) * 128
                    qt_t = qt2[i // 4]
                    qsl = np.s_[:, (i % 4) * 128:(i % 4) * 128 + 128]
                    for lo in range(0, e, 512):
                        w = min(512, e - lo)
                        nc.tensor.matmul(pl_ev[:, lo // 128:(lo + w) // 128, :],
                                         lhsT=qt_t[qsl],
                                         rhs=kt4[(0, lo // 512)][:, 0:w],
                                         start=True, stop=True)
                    # the last even chunk is the only mask-dependent one
                    # (diagonal for h=0 cores, fully visible for h=1)
                    nc.vector.tensor_add(pl_ev[:, i, :], pl_ev[:, i, :],
                                         mA_sb[:])
                    nmo = stat.tile([128, 1], F32, tag="nmo")
                    nc.vector.reduce_max(out=nmo[:], in_=pl_ev[:, :i + 1, :],
                                         axis=mybir.AxisListType.XY, negate=True)
                    lm = lmpool.tile([128, S], BF16, tag="lm")
                    nc.scalar.copy(lm[:, :e], pl_ev[:, :i + 1, :])
                    for lo in range(0, e, 512):
                        w = min(512, e - lo)
                        nc.tensor.matmul(pl_od[:, lo // 128:(lo + w) // 128, :],
                                         lhsT=qt_t[qsl],
                                         rhs=kt4[(1, lo // 512)][:, 0:w],
                                         start=True, stop=True)
                    # ... and the last odd chunk (masked for h=0, diag for h=1)
                    nc.vector.tensor_add(pl_od[:, i, :], pl_od[:, i, :],
                                         mB_sb[:])
                    nmp = stat.tile([128, 1], F32, tag="nmp")
                    nc.vector.reduce_max(out=nmp[:], in_=pl_od[:, :i + 1, :],
                                         axis=mybir.AxisListType.XY, negate=True)
                    # combined -(row max), rounded to bf16: RNE is monotone, so
                    # this is exactly -(max of the bf16 logits)
                    nmb = stat.tile([128, 1], BF16, tag="nmb")
                    nc.vector.tensor_tensor(nmb[:], nmo[:], nmp[:],
                                            op=mybir.AluOpType.min)
                    nmf = stat.tile([128, 1], F32, tag="nmf")
                    nc.vector.tensor_copy(nmf[:], nmb[:])
                    nc.vector.tensor_copy(lm[:, e:2 * e], pl_od[:, :i + 1, :])
                    tile_state[i] = (lm, nmf)

                def stage2(i):
                    e = (i + 1) * 128
                    lm, nmf = tile_state.pop(i)
                    p = i // 2
                    if i % 2 == 0:
                        wt = wtpool.tile([128, 16, 256], BF16, tag="wt")
                        pair_bufs[p] = wt
                        # slots the even tile doesn't cover (its half only)
                        nc.gpsimd.memset(wt[:, i + 1, 0:128], 0.0)
                        nc.gpsimd.memset(wt[:, 9 + i, 0:128], 0.0)
                    else:
                        wt = pair_bufs[p]
                    half = np.s_[(i % 2) * 128:(i % 2) * 128 + 128]

                    w_t = wepool.tile([128, S], BF16, tag="w")
                    nc.scalar.activation(
                        out=w_t[:, :2 * e], in_=lm[:, :2 * e],
                        func=mybir.ActivationFunctionType.Exp,
                        bias=nmf[:], scale=1.0,
                        accum_out=sums_sb[:, i:i + 1])

                    nc.sync.dma_start_transpose(out=wt[:, 0:i + 1, half],
                                                in_=w_t[:, :e])
                    nc.sync.dma_start_transpose(out=wt[:, 8:9 + i, half],
                                                in_=w_t[:, e:2 * e])

                def pv(p):
                    wt = pair_bufs.pop(p)
                    hi = 2 * p + 1
                    slots = list(range(0, hi + 1)) + list(range(8, 9 + hi))
                    po = popool.tile([128, 256], F32, tag="poT")
                    for n, s in enumerate(slots):
                        r, ls = s // 8, s % 8
                        vt_t = v4[(r, ls // 4)]
                        nc.tensor.matmul(po[:], lhsT=vt_t[:, ls % 4, :],
                                         rhs=wt[:, s, :],
                                         start=(n == 0), stop=(n == len(slots) - 1))
                    o_sb = opool.tile([128, 256], BF16, tag="o")
                    nc.vector.tensor_copy(o_sb[:], po[:])
                    nc.gpsimd.dma_start(out=outT[:, p * 256:(p + 1) * 256],
                                        in_=o_sb[:])

                # lag-1 schedule: stage1 runs one tile ahead of stage2
                stage1(0)
                for i in range(1, QT):
                    stage1(i)
                    stage2(i - 1)
                    if i % 2 == 0 and i >= 2:
                        pv(i // 2 - 1)
                stage2(QT - 1)
                pv(3)
                nc.gpsimd.dma_start(out=sums[:], in_=sums_sb[:])

    nc.finalize()
    return nc


def shard_inputs(x, attn_mask, Wq, bq, Wk, bk, Wv, bv):
    """Host-side shard prep. Returns in_maps for cores 0..7."""
    bf = ml_dtypes.bfloat16
    xb = np.asarray(x).astype(bf)                   # cast first, like the reference
    mask_f = np.asarray(attn_mask)

    def tile_w(W):
        # [DH, D] -> [128, D_CH, DH] with w[p, i, m] = W[m, i*128+p]
        WT = np.asarray(W).astype(bf).T.reshape(D_CH, 128, DH)
        return np.ascontiguousarray(WT.transpose(1, 0, 2))

    wqt, wkt, wvt = tile_w(Wq), tile_w(Wk), tile_w(Wv)
    bqc = np.asarray(bq).astype(bf).astype(np.float32).reshape(DH, 1)
    bkc = np.asarray(bk).astype(bf).astype(np.float32).reshape(DH, 1)

    in_maps = []
    for c in range(N_CORES):
        b, h = divmod(c, 2)
        own = np.concatenate([np.arange(t * 128, (t + 1) * 128)
                              for t in range(h, 16, 2)])
        xT = np.ascontiguousarray(xb[b][own].T)                      # [D, SQ]
        # by causal structure every (q-tile, last even/odd chunk) mask block
        # equals the corresponding block in the first two tile rows
        mA = np.ascontiguousarray(mask_f[h * 128:(h + 1) * 128, 0:128].astype(bf))
        mB = np.ascontiguousarray(mask_f[h * 128:(h + 1) * 128, 128:256].astype(bf))
        in_maps.append({
            "xT": xT, "maskA": mA, "maskB": mB,
            "wqT": wqt, "wkT": wkt, "wvT": wvt,
            "bq": bqc, "bk": bkc,
        })
    return in_maps


def unshard(core_out, bv):
    """core_out: dicts with 'outT' [DH, SQ] bf16, 'sums' [128, QT] f32."""
    bvf = np.asarray(bv).astype(ml_dtypes.bfloat16).astype(np.float32)
    out = np.empty((B, S, DH), dtype=ml_dtypes.bfloat16)
    for c in range(N_CORES):
        b, h = divmod(c, 2)
        oT = np.asarray(core_out[c]["outT"], dtype=np.float32)
        sm = np.asarray(core_out[c]["sums"], dtype=np.float32)
        for j in range(QT):
            t = h + 2 * j
            blk = oT[:, j * 128:(j + 1) * 128] / sm[:, j][None, :]
            out[b, t * 128:(t + 1) * 128, :] = \
                (blk.T + bvf[None, :]).astype(ml_dtypes.bfloat16)
    return out


_NC_CACHE = {}


def kernel(x, attn_mask, Wq, bq, Wk, bk, Wv, bv):
    if "nc" not in _NC_CACHE:
        _NC_CACHE["nc"] = build_nc()
    nc = _NC_CACHE["nc"]
    in_maps = shard_inputs(x, attn_mask, Wq, bq, Wk, bk, Wv, bv)
    res = run_bass_kernel_spmd(nc, in_maps, list(range(N_CORES)))
    return unshard(res.results, bv)
